# revision 14
# baseline (speedup 1.0000x reference)
"""Trainium2 Bass kernel for nn_BasicNCA2D_GroupEq (group-equivariant NCA).

Self-contained: accepts FULL inputs, shards batch across 8 NeuronCores
(1 image per core), runs all `steps` NCA iterations in a single NEFF with
a per-step 512-byte AllGather for the global BatchNorm statistics, and
gathers the full output.

Key restructurings (validated numerically against the reference):
  * Z2->P4 7x7 conv as 7 accumulating matmuls per 2-row tile with
    K=(row-offset,in-channel)=112 on the PE partition axis; the 7 column
    shifts (dx) are pure access-pattern offsets into a 198-pitched,
    7x row-shifted replica buffer X7 (seam columns hold zeros so SAME
    padding falls out of the arithmetic).
  * The P4->P4 1x1 conv followed by mean-over-rotations collapses exactly
    to a single 64->16 channel matmul (the rolled weight sums out).
  * conv1's bias b1 cancels inside BatchNorm and is dropped.
  * BatchNorm batch statistics via hardware bn_stats/bn_aggr + one
    AllGather of per-core (sum, sumsq) per channel.
  * The stochastic update mask is jax.random-generated on the host with
    the default backend so the bits match the reference's PRNG stream.
"""

import numpy as np

B, H, W, C = 8, 192, 192, 16
CH = 16
O4 = 4 * CH          # 64 p4 feature channels (o*4+r)
K7 = 7
EPS = 1e-5
FIRE_RATE = 0.5
PITCH = W + 6        # 198: row pitch in X7 (6 zero seam columns)
XCOLS = 3 + H * PITCH + 3
NPIX = H * W
ROWT = 2             # image rows per conv1 psum tile
NT1 = H // ROWT      # 96 conv1 tiles
BAND = 6             # image rows per phase-B band
NBAND = H // BAND    # 32
N_CORES = 8
NTOT = float(B * 4 * H * W)   # batchnorm normalization count
DY_ORDER = (3, 0, 1, 2, 4, 5, 6)   # X7 partition-slot order (dy=3 at base 0)
DY_SLOT = {dy: i for i, dy in enumerate(DY_ORDER)}

_CACHE = {}


def _build(steps, use_f32r=True):
    import concourse.bacc as bacc
    import concourse.bass as bass
    import concourse.tile as tile
    from concourse import mybir
    from contextlib import ExitStack

    F32 = mybir.dt.float32
    MMDT = mybir.dt.float32r if use_f32r else F32

    nc = bacc.Bacc("TRN2", target_bir_lowering=False, debug=False,
                   num_devices=N_CORES)

    x7_d = nc.dram_tensor("x7", [K7 * C, XCOLS], F32, kind="ExternalInput").ap()
    w1l_d = nc.dram_tensor("w1l", [K7, K7 * C, O4], F32, kind="ExternalInput").ap()
    l2_d = nc.dram_tensor("l2", [O4, CH], F32, kind="ExternalInput").ap()
    g2_d = nc.dram_tensor("g2", [O4, O4], F32, kind="ExternalInput").ap()
    bnv_d = nc.dram_tensor("bnv", [O4, 3], F32, kind="ExternalInput").ap()
    if steps > 0:
        mask_d = nc.dram_tensor("masks", [steps, NPIX], F32, kind="ExternalInput").ap()
    out_d = nc.dram_tensor("out", [C, NPIX], F32, kind="ExternalOutput").ap()

    def cast_dma(out, in_):
        # SWDGE dma casts f32 -> f32r (rounding) when dtypes differ
        nc.gpsimd.dma_start(out=out, in_=in_)

    with tile.TileContext(nc) as tc, ExitStack() as ctx:
        big = ctx.enter_context(tc.tile_pool(name="big", bufs=1))
        consts = ctx.enter_context(tc.tile_pool(name="consts", bufs=1))
        ps1p = ctx.enter_context(tc.tile_pool(name="ps1", bufs=4, space="PSUM"))
        ps2p = ctx.enter_context(tc.tile_pool(name="ps2", bufs=2, space="PSUM"))
        psgp = ctx.enter_context(tc.tile_pool(name="psg", bufs=2, space="PSUM"))
        bnp = ctx.enter_context(tc.tile_pool(name="bn", bufs=1))
        vec = ctx.enter_context(tc.tile_pool(name="vec", bufs=2))
        yb = ctx.enter_context(tc.tile_pool(name="yb", bufs=3))
        tb = ctx.enter_context(tc.tile_pool(name="tb", bufs=2))
        ydram = ctx.enter_context(tc.tile_pool(name="ydram", bufs=2, space="DRAM"))
        ccd = ctx.enter_context(tc.tile_pool(name="ccd", bufs=2, space="DRAM"))

        # persistent state + constants
        X7 = big.tile([K7 * C, XCOLS], MMDT)
        cast_dma(X7[:], x7_d[:])
        w1s = []
        for dx in range(K7):
            wt = consts.tile([K7 * C, O4], MMDT, tag=f"w1_{dx}")
            cast_dma(wt[:], w1l_d[dx])
            w1s.append(wt)
        l2s = consts.tile([O4, CH], MMDT, tag="l2")
        cast_dma(l2s[:], l2_d[:])
        g2s = consts.tile([O4, O4], F32, tag="g2")
        nc.sync.dma_start(out=g2s[:], in_=g2_d[:])
        bnv = consts.tile([O4, 3], F32, tag="bnv")
        nc.sync.dma_start(out=bnv[:], in_=bnv_d[:])
        gamma64 = bnv[:, 0:1]
        beta64 = bnv[:, 1:2]
        b2_16 = bnv[0:CH, 2:3]
        epst = consts.tile([O4, 1], F32, tag="eps")
        nc.vector.memset(epst[:], EPS)

        for s in range(steps):
            # ---------------- phase A: conv1 + stats ----------------
            y_d = ydram.tile([O4, NPIX], F32)
            bnbuf = bnp.tile([O4, NT1, 6], F32)
            TPB = BAND // ROWT  # conv1 tiles per staging band
            for bb in range(NBAND):
                ystage = yb.tile([O4, BAND * W], F32, tag="ybig")
                for q in range(TPB):
                    t = bb * TPB + q
                    ps1 = ps1p.tile([O4, ROWT * PITCH], F32)
                    for j, dx in enumerate((3, 0, 1, 2, 4, 5, 6)):
                        c0 = ROWT * t * PITCH + dx
                        nc.tensor.matmul(
                            ps1[:], w1s[dx][:], X7[:, c0:c0 + ROWT * PITCH],
                            start=(j == 0), stop=(j == K7 - 1),
                        )
                    v3 = ps1[:].rearrange("p (h w) -> p h w", w=PITCH)[:, :, 0:W]
                    ysl = ystage[:, q * ROWT * W:(q + 1) * ROWT * W]
                    nc.scalar.copy(out=ysl.rearrange("p (h w) -> p h w", w=W), in_=v3)
                    nc.vector.bn_stats(out=bnbuf[:, t:t + 1, :], in_=ysl)
                nc.sync.dma_start(
                    out=y_d[:, bb * BAND * W:(bb + 1) * BAND * W], in_=ystage[:])

            # aggregate stats -> (sum, sumsq) -> allgather -> a/bb vectors
            mv = vec.tile([O4, 2], F32, tag="mv")
            nc.vector.bn_aggr(out=mv[:], in_=bnbuf[:])
            ccs = vec.tile([O4, 2], F32, tag="ccs")
            # ccs0 = mean*NPIX ; ccs1 = (var + mean^2)*NPIX
            nc.vector.tensor_mul(ccs[:, 1:2], mv[:, 0:1], mv[:, 0:1])
            nc.vector.tensor_add(ccs[:, 1:2], ccs[:, 1:2], mv[:, 1:2])
            nc.vector.tensor_scalar_mul(ccs[:, 1:2], in0=ccs[:, 1:2], scalar1=float(NPIX))
            nc.vector.tensor_scalar_mul(ccs[:, 0:1], in0=mv[:, 0:1], scalar1=float(NPIX))
            cc_in = ccd.tile([O4, 2], F32, tag="cc_in")
            cc_out = ccd.tile([N_CORES * O4, 2], F32, addr_space="Shared", tag="cc_out")
            nc.sync.dma_start(out=cc_in[:], in_=ccs[:])
            nc.gpsimd.collective_compute(
                "AllGather", mybir.AluOpType.bypass,
                ins=[cc_in[:]], outs=[cc_out[:]],
                replica_groups=[list(range(N_CORES))],
            )
            agg = vec.tile([O4, 2, N_CORES], F32, tag="agg")
            cc_ap = cc_out[:]
            src = bass.AP(tensor=cc_ap.tensor, offset=cc_ap.offset,
                          ap=[[2, O4], [1, 2], [2 * O4, N_CORES]])
            nc.sync.dma_start(out=agg[:], in_=src)
            asum = vec.tile([O4, 2], F32, tag="asum")
            nc.vector.tensor_reduce(out=asum[:], in_=agg[:],
                                    axis=mybir.AxisListType.X,
                                    op=mybir.AluOpType.add)
            psg = psgp.tile([O4, 2], F32)
            nc.tensor.matmul(psg[:], g2s[:], asum[:], start=True, stop=True)
            stats = vec.tile([O4, 4], F32, tag="stats")  # mu, var, a, bb
            mu = stats[:, 0:1]
            var = stats[:, 1:2]
            av = stats[:, 2:3]
            bbv = stats[:, 3:4]
            nc.vector.tensor_scalar_mul(mu, in0=psg[:, 0:1], scalar1=1.0 / NTOT)
            nc.vector.tensor_scalar_mul(var, in0=psg[:, 1:2], scalar1=1.0 / NTOT)
            musq = vec.tile([O4, 1], F32, tag="musq")
            nc.vector.tensor_mul(musq[:], mu, mu)
            nc.vector.tensor_sub(var, var, musq[:])
            sd = vec.tile([O4, 1], F32, tag="sd")
            nc.scalar.activation(out=sd[:], in_=var,
                                 func=mybir.ActivationFunctionType.Sqrt,
                                 bias=epst[:], scale=1.0)
            nc.vector.reciprocal(out=sd[:], in_=sd[:])
            nc.vector.tensor_mul(av, gamma64, sd[:])
            nc.vector.tensor_mul(musq[:], av, mu)
            nc.vector.tensor_sub(bbv, beta64, musq[:])

            # ---------------- phase B: norm+relu, conv2, update ----------------
            for b in range(NBAND):
                z0 = b * BAND
                cs = z0 * W
                yin = yb.tile([O4, BAND * W], F32, tag="ybig")
                nc.sync.dma_start(out=yin[:], in_=y_d[:, cs:cs + BAND * W])
                yr = yb.tile([O4, BAND * W], MMDT, tag="ybig")
                nc.scalar.activation(out=yr[:], in_=yin[:],
                                     func=mybir.ActivationFunctionType.Relu,
                                     bias=bbv, scale=av)
                bt = tb.tile([32 + C, BAND * W], F32, tag="bt")
                msk = bt[0:C]
                tmp = tb.tile([C, BAND * W], F32, tag="tmp")
                msrc = bass.AP(tensor=mask_d.tensor,
                               offset=mask_d.offset + s * NPIX + cs,
                               ap=[[0, C], [1, BAND * W]])
                nc.gpsimd.dma_start(out=msk[:], in_=msrc)

                NQ = BAND // 2
                for q in range(NQ):
                    ps2 = ps2p.tile([CH, ROWT * W], F32)
                    nc.tensor.matmul(ps2[:], l2s[:],
                                     yr[:, q * ROWT * W:(q + 1) * ROWT * W],
                                     start=True, stop=True)
                    nc.vector.scalar_tensor_tensor(
                        out=tmp[:, q * ROWT * W:(q + 1) * ROWT * W],
                        in0=ps2[:], scalar=b2_16,
                        in1=msk[:, q * ROWT * W:(q + 1) * ROWT * W],
                        op0=mybir.AluOpType.add, op1=mybir.AluOpType.mult)
                xnew = bt[32:32 + C]
                xold = X7[0:C, 3 + z0 * PITCH: 3 + (z0 + BAND) * PITCH]
                xold = xold.rearrange("p (h w) -> p h w", w=PITCH)[:, :, 0:W]
                if use_f32r:
                    xold = xold.bitcast(F32)
                nc.vector.tensor_add(
                    out=xnew[0:C].rearrange("p (h w) -> p h w", w=W),
                    in0=xold,
                    in1=tmp[0:C].rearrange("p (h w) -> p h w", w=W))
                # replicate updated rows into all 7 dy-shifted slots of X7
                for dy in range(K7):
                    rr_lo = max(0, z0 + 3 - dy)
                    rr_hi = min(H, z0 + BAND + 3 - dy)
                    n = rr_hi - rr_lo
                    if n <= 0:
                        continue
                    sl = rr_lo + dy - 3 - z0
                    sb0 = DY_SLOT[dy] * C
                    dst = X7[sb0 + 1:sb0 + C,
                             3 + rr_lo * PITCH: 3 + rr_hi * PITCH]
                    dst = dst.rearrange("p (h w) -> p h w", w=PITCH)[:, :, 0:W]
                    src2 = xnew[1:C, sl * W:(sl + n) * W]
                    cast_dma(dst, src2.rearrange("p (h w) -> p h w", w=W))

        # final output: X7 dy=3 slot (canonical x) -> DRAM
        xout = X7[0:C, 3: 3 + H * PITCH]
        xout = xout.rearrange("p (h w) -> p h w", w=PITCH)[:, :, 0:W]
        if use_f32r:
            xout = xout.bitcast(F32)
        nc.sync.dma_start(out=out_d[:].rearrange("p (h w) -> p h w", w=W),
                          in_=xout)

    nc.compile()
    return nc


def _host_inputs(x, w1, w2, b2, masks):
    """Per-core input maps. x: [B,H,W,C] f32, masks: [steps,B,H,W] f32."""
    ws = np.stack([np.rot90(w1, k=r, axes=(2, 3)) for r in range(4)], axis=1)
    w1l_dy = np.transpose(ws, (4, 3, 2, 0, 1)).reshape(K7, K7, C, O4)  # [dx, dy, i, c]
    w1l = np.ascontiguousarray(
        w1l_dy[:, list(DY_ORDER)].reshape(K7, K7 * C, O4)).astype(np.float32)
    m2 = 0.25 * w2[:, :, :, 0, 0].sum(axis=2)          # [O,I]
    l2 = np.ascontiguousarray(np.repeat(m2.T, 4, axis=0)).astype(np.float32)  # [64,16]
    g2 = np.zeros((O4, O4), dtype=np.float32)
    for c in range(O4):
        g2[c, (c // 4) * 4:(c // 4) * 4 + 4] = 1.0
    return w1l, l2, g2


def kernel(x, w1, b1, gamma, beta, w2, b2, steps):
    x = np.asarray(x, dtype=np.float32)
    w1 = np.asarray(w1, dtype=np.float32)
    gamma = np.asarray(gamma, dtype=np.float32)
    beta = np.asarray(beta, dtype=np.float32)
    w2 = np.asarray(w2, dtype=np.float32)
    b2 = np.asarray(b2, dtype=np.float32)
    steps = int(steps)
    if steps <= 0:
        return x.copy()

    import os
    use_f32r = os.environ.get("KERNEL_F32R", "1") == "1"

    # masks must reproduce the reference's jax PRNG stream bit-exactly:
    # same ops on the default backend (rbg bits are backend-dependent).
    import jax
    import jax.numpy as jnp
    base = jax.random.key(42)
    masks = np.empty((steps, B, H, W), dtype=np.float32)
    for s in range(steps):
        k = jax.random.fold_in(base, s)
        u = np.asarray(jax.random.uniform(k, (B, 1, H, W), dtype=jnp.float32))
        masks[s] = (u[:, 0] > FIRE_RATE).astype(np.float32)

    w1l, l2, g2 = _host_inputs(x, w1, w2, b2, masks)
    bnv = np.zeros((O4, 3), dtype=np.float32)
    bnv[:, 0] = np.repeat(gamma, 4)
    bnv[:, 1] = np.repeat(beta, 4)
    bnv[:CH, 2] = b2

    key = (steps, use_f32r)
    if key not in _CACHE:
        _CACHE[key] = _build(steps, use_f32r)
    nc = _CACHE[key]

    in_maps = []
    for bi in range(B):
        xc = np.ascontiguousarray(np.transpose(x[bi], (2, 0, 1)))  # [C,H,W]
        x7 = np.zeros((K7, C, XCOLS), dtype=np.float32)
        body = np.zeros((C, H, PITCH), dtype=np.float32)
        for slot, dy in enumerate(DY_ORDER):
            body[:] = 0.0
            lo, hi = max(0, 3 - dy), min(H, H + 3 - dy)
            body[:, lo:hi, 0:W] = xc[:, lo + dy - 3:hi + dy - 3, :]
            x7[slot, :, 3:3 + H * PITCH] = body.reshape(C, H * PITCH)
        im = {
            "x7": x7.reshape(K7 * C, XCOLS),
            "w1l": w1l, "l2": l2, "g2": g2, "bnv": bnv,
            "masks": np.ascontiguousarray(masks[:, bi].reshape(steps, NPIX)),
        }
        in_maps.append(im)

    from concourse.bass_utils import run_bass_kernel_spmd
    trace = os.environ.get("KERNEL_TRACE", "0") == "1"
    res = run_bass_kernel_spmd(nc, in_maps, core_ids=list(range(N_CORES)),
                               trace=trace)
    if trace and res.exec_time_ns is not None:
        print(f"HW exec time: {res.exec_time_ns} ns")
        kernel.last_exec_time_ns = res.exec_time_ns
    kernel.last_results = res

    out = np.empty((B, H, W, C), dtype=np.float32)
    for bi in range(B):
        o = res.results[bi]["out"].reshape(C, H, W)
        out[bi] = np.transpose(o, (1, 2, 0))
    return out
